# revision 13
# baseline (speedup 1.0000x reference)
"""MoE gate (router) kernel for Trainium2, 8 NeuronCores.

Computes, for hidden_states [4, 4096, 2048] f32 and router weight [64, 2048] f32:
  logits = x @ w.T -> softmax over 64 experts -> top-2 (+ normalized weights)
  plus the seq_aux load-balancing loss.

Sharding: tokens (batch*seq = 16384) split contiguously across 8 cores
(2048 each); router weight replicated. Host pre-tiles each token shard into
the exact SBUF tile layout (group-major 512-token groups, 2-chunk DMA
pieces, bf16 hi/lo halves fused) so every DMA is a pure contiguous copy.
The bf16 hi/lo split gives the router GEMM bf16 PE rate at ~fp32 accuracy
(products are exact; only the fp32 psum accumulate rounds).

Kernel structure (per core): 512-token groups are pipelined - group g's
softmax/top-2 (DVE/ACT) overlaps group g+1's GEMM (PE). x loads alternate
between the two HWDGE rings (sync + scalar); outputs ride SWDGE (gpsimd).
A burst of dummy tiny matmuls at kernel start warms the PE clock (HAM)
during the first DMA wait.
"""

import os
import numpy as np
import ml_dtypes

import concourse.bass as bass
import concourse.mybir as mybir
import concourse.tile as tile
from concourse import bacc
from concourse.bass_utils import run_bass_kernel_spmd

F32 = mybir.dt.float32
BF16 = mybir.dt.bfloat16
I32 = mybir.dt.int32
U32 = mybir.dt.uint32
NP_BF16 = ml_dtypes.bfloat16

N_CORES = 8
BSZ = 4
SEQ = 4096
DIM = 2048
E = 64
TOP_K = 2
ALPHA = 0.01
N_TOKENS = BSZ * SEQ                 # 16384
TOK_PER_CORE = N_TOKENS // N_CORES   # 2048
P = 128
N_TILES = TOK_PER_CORE // P          # 16 token tiles per core
H_CHUNKS = DIM // P                  # 16 contraction chunks
N_GROUPS = 4                         # 512-token matmul groups
GTOK = TOK_PER_CORE // N_GROUPS      # 512
TILES_PER_GROUP = GTOK // P          # 4
KSPLIT = 2                           # h-chunks per x DMA piece
N_K = H_CHUNKS // KSPLIT             # 8 DMA pieces per group (per shard)

MODE = os.environ.get("BASS_MOE_MODE", "bf16x3")
N_WARMUP = int(os.environ.get("BASS_MOE_WARMUP", "18"))


def _softmax_top2(nc, soft, pl, t, ex_all, rs_all, w_all, idx_all):
    """pl: PSUM [128, 64] logits for token tile t. Emits softmax+top2."""
    negmax = soft.tile([P, 1], F32, tag="negmax")
    nc.vector.tensor_reduce(
        negmax, pl, axis=mybir.AxisListType.X,
        op=mybir.AluOpType.max, negate=True,
    )
    ex_t = ex_all[:, t * E:(t + 1) * E]
    sumex = soft.tile([P, 1], F32, tag="sumex")
    nc.scalar.activation(
        ex_t, pl, mybir.ActivationFunctionType.Exp,
        bias=negmax, scale=1.0, accum_out=sumex,
    )
    nc.vector.reciprocal(rs_all[:, t:t + 1], sumex)

    m8 = soft.tile([P, 8], F32, tag="m8")
    nc.vector.max(out=m8, in_=ex_t)
    nc.vector.max_index(out=idx_all[:, 8 * t:8 * t + 8], in_max=m8,
                        in_values=ex_t)

    # m8[:,0] == 1.0 exactly (exp(max-max)); s12 = 1 + e2
    s12 = soft.tile([P, 1], F32, tag="s12")
    nc.vector.tensor_scalar_add(s12, m8[:, 1:2], 1.0)
    rs12 = soft.tile([P, 1], F32, tag="rs12")
    nc.vector.reciprocal(rs12, s12)
    nc.vector.tensor_scalar_mul(w_all[:, 2 * t:2 * t + 2], m8[:, 0:2], rs12)


def build_program(mode=MODE):
    nc = bacc.Bacc("TRN2", target_bir_lowering=False)

    if mode == "f32":
        xdt, wdt = F32, F32
        n_split = 1
        w_names = ["wt"]
        terms = [(0, 0)]
    else:
        xdt, wdt = BF16, BF16
        n_split = 2
        w_names = ["wth", "wtl"]
        terms = [(0, 0), (0, 1), (1, 0)]
        if mode == "bf16x4":
            terms.append((1, 1))

    # pre-tiled x: [group, kpiece, split, partition, chunk-in-piece, token]
    x_in = nc.dram_tensor("x", [N_GROUPS, N_K, n_split, P, KSPLIT, GTOK], xdt,
                          kind="ExternalInput")
    wt_ins = [nc.dram_tensor(n, [P, H_CHUNKS, E], wdt, kind="ExternalInput")
              for n in w_names]
    ident_in = nc.dram_tensor("ident", [P, P], F32, kind="ExternalInput")

    out_w = nc.dram_tensor("out_w", [P, 2 * N_TILES], F32, kind="ExternalOutput")
    out_idx = nc.dram_tensor("out_idx", [P, 8 * N_TILES], U32,
                             kind="ExternalOutput")
    out_ssum = nc.dram_tensor("out_ssum", [N_TILES, TILES_PER_GROUP * E], F32,
                              kind="ExternalOutput")

    with tile.TileContext(nc) as tc:
        with (
            tc.tile_pool(name="consts", bufs=1) as consts,
            tc.tile_pool(name="xload", bufs=2) as xload,
            tc.tile_pool(name="lgsb", bufs=2) as lgsb,
            tc.tile_pool(name="soft", bufs=4) as soft,
            tc.tile_pool(name="acc", bufs=1) as acc,
            tc.tile_pool(name="pwu", bufs=1, space="PSUM") as pwu,
            tc.tile_pool(name="plg", bufs=2, space="PSUM") as plg,
            tc.tile_pool(name="plp", bufs=2, space="PSUM") as plp,
            tc.tile_pool(name="pst", bufs=2, space="PSUM") as pst,
        ):
            # ---- PE warmup: no-dep matmuls while first DMAs land ----
            # ~N_WARMUP x 213ns of PE busy; they sit in the PE's 64-deep
            # queue so they keep running even when the sequencer blocks on
            # the first data-dependent matmul.
            if N_WARMUP:
                wu_sb = consts.tile([P, 512], BF16)
                nc.vector.memset(wu_sb, 0.0)
                wu_ps = pwu.tile([8, 512], F32)
                for _ in range(N_WARMUP):
                    nc.tensor.matmul(wu_ps, lhsT=wu_sb[:, :8], rhs=wu_sb,
                                     start=True, stop=True,
                                     skip_group_check=True)

            # weights ride the scalar (ACT) HWDGE ring, host-pretiled so the
            # DMA is a contiguous copy; identity loads first on the sync ring
            ident_sb = consts.tile([P, P], F32)
            nc.sync.dma_start(ident_sb, ident_in[:, :])
            wt_sbs = []
            for wi, wt_in in enumerate(wt_ins):
                wsb = consts.tile([P, H_CHUNKS, E], wdt, tag=f"wt{wi}",
                                  name=f"wt{wi}")
                nc.scalar.dma_start(wsb, wt_in[:, :, :])
                wt_sbs.append(wsb)

            ex_all = acc.tile([P, N_TILES * E], F32)
            rs_all = acc.tile([P, N_TILES], F32)
            w_all = acc.tile([P, 2 * N_TILES], F32)
            idx_all = acc.tile([P, 8 * N_TILES], U32)

            lgT_live = {}

            def emit_gemm(g, dma_i):
                # this group's x: N_K pieces, alternating HWDGE rings
                xg = []
                for k in range(N_K):
                    xk = xload.tile([P, n_split, KSPLIT, GTOK], xdt,
                                    tag=f"xk{k}", name=f"xk{k}")
                    eng = nc.sync if (dma_i % 2 == 0) else nc.scalar
                    eng.dma_start(
                        xk, x_in[g, k].rearrange("h p j t -> p h j t"))
                    dma_i += 1
                    xg.append(xk)

                lgT = plg.tile([E, GTOK], F32, tag="lgT")
                for c in range(H_CHUNKS):
                    k, j = divmod(c, KSPLIT)
                    for ti, (wi, xi) in enumerate(terms):
                        nc.tensor.matmul(
                            lgT,
                            lhsT=wt_sbs[wi][:, c, :],
                            rhs=xg[k][:, xi, j, :],
                            start=(c == 0 and ti == 0),
                            stop=(c == H_CHUNKS - 1 and ti == len(terms) - 1),
                            skip_group_check=True,
                        )
                lgT_live[g] = lgT
                return dma_i

            def emit_softmax(g):
                # logits^T [64, 512] -> sbuf -> 4x transpose -> softmax
                lgT_sb = lgsb.tile([E, GTOK], F32, tag="lgT_sb")
                nc.scalar.copy(lgT_sb, lgT_live.pop(g))
                for j in range(TILES_PER_GROUP):
                    t = g * TILES_PER_GROUP + j
                    pl = plp.tile([P, E], F32, tag="logits")
                    nc.tensor.transpose(
                        pl, lgT_sb[:, j * P:(j + 1) * P], ident_sb[:E, :E],
                    )
                    _softmax_top2(nc, soft, pl, t, ex_all, rs_all, w_all,
                                  idx_all)

                # group aux-loss partials (diag-block trick):
                # out[m, :] = sum_tok rs[tok, g*4+m] * ex[tok, :]
                ps = pst.tile([TILES_PER_GROUP, TILES_PER_GROUP * E], F32,
                              tag="stats")
                nc.tensor.matmul(
                    ps,
                    lhsT=rs_all[:, g * TILES_PER_GROUP:(g + 1) * TILES_PER_GROUP],
                    rhs=ex_all[:, g * TILES_PER_GROUP * E:
                               (g + 1) * TILES_PER_GROUP * E],
                    start=True, stop=True,
                    skip_group_check=True,
                )
                ss = soft.tile([TILES_PER_GROUP, TILES_PER_GROUP * E], F32,
                               tag="ss")
                nc.scalar.copy(ss, ps)
                nc.sync.dma_start(
                    out_ssum[g * TILES_PER_GROUP:(g + 1) * TILES_PER_GROUP, :],
                    ss)

            # software pipeline: group g's softmax is emitted after group
            # g+1's GEMM so the PE stream never stalls on the lgT copy
            dma_i = 0
            for g in range(N_GROUPS):
                dma_i = emit_gemm(g, dma_i)
                if g >= 1:
                    emit_softmax(g - 1)
            emit_softmax(N_GROUPS - 1)

            nc.sync.dma_start(out_w[:, :], w_all[:, :])
            nc.sync.dma_start(out_idx[:, :], idx_all[:, :])

    if not nc.is_finalized():
        nc.finalize()
    return nc


_NC = {}


def _get_nc(mode=MODE):
    if mode not in _NC:
        _NC[mode] = build_program(mode)
    return _NC[mode]


def _pretile(splits):
    """splits: list of [2048 tok, 2048 dim] arrays (hi[, lo]).
    -> [N_GROUPS, N_K, n_split, P, KSPLIT, GTOK] contiguous."""
    # (s, g, t, k, j, p) -> (g, k, s, p, j, t)
    st = np.stack(
        [s.reshape(N_GROUPS, GTOK, N_K, KSPLIT, P) for s in splits], axis=0)
    return np.ascontiguousarray(st.transpose(1, 3, 0, 5, 4, 2))


def _prep_inputs(hidden_states, weight, mode=MODE):
    x = np.asarray(hidden_states, dtype=np.float32).reshape(N_TOKENS, DIM)
    w = np.asarray(weight, dtype=np.float32)
    ident = np.eye(P, dtype=np.float32)

    in_maps = []
    if mode == "f32":
        wt = np.ascontiguousarray(
            w.T.reshape(H_CHUNKS, P, E).transpose(1, 0, 2))
        for c in range(N_CORES):
            xs = x[c * TOK_PER_CORE:(c + 1) * TOK_PER_CORE, :]
            in_maps.append({"x": _pretile([xs]), "wt": wt, "ident": ident})
    else:
        wh = w.astype(NP_BF16)
        wl = (w - wh.astype(np.float32)).astype(NP_BF16)
        wth = np.ascontiguousarray(
            wh.T.reshape(H_CHUNKS, P, E).transpose(1, 0, 2))
        wtl = np.ascontiguousarray(
            wl.T.reshape(H_CHUNKS, P, E).transpose(1, 0, 2))
        for c in range(N_CORES):
            xs = x[c * TOK_PER_CORE:(c + 1) * TOK_PER_CORE, :]
            xh = xs.astype(NP_BF16)
            xl = (xs - xh.astype(np.float32)).astype(NP_BF16)
            in_maps.append({
                "x": _pretile([xh, xl]),
                "wth": wth, "wtl": wtl, "ident": ident,
            })
    return in_maps


def _run(hidden_states, weight, trace=False, mode=MODE, tmpdir=None):
    in_maps = _prep_inputs(hidden_states, weight, mode)
    nc = _get_nc(mode)
    kw = {}
    if trace:
        kw["trace"] = True
        if tmpdir:
            kw["tmpdir"] = tmpdir
    return run_bass_kernel_spmd(nc, in_maps, list(range(N_CORES)), **kw)


def _assemble(results):
    topk_w = np.empty((N_TOKENS, TOP_K), dtype=np.float32)
    topk_idx = np.empty((N_TOKENS, TOP_K), dtype=np.int32)
    ssum = np.empty((N_CORES, E), dtype=np.float32)
    for c, res in enumerate(results):
        wv = res["out_w"].reshape(P, N_TILES, TOP_K)
        iv = res["out_idx"].reshape(P, N_TILES, 8)[:, :, :TOP_K]
        base = c * TOK_PER_CORE
        topk_w[base:base + TOK_PER_CORE] = (
            wv.transpose(1, 0, 2).reshape(TOK_PER_CORE, TOP_K))
        topk_idx[base:base + TOK_PER_CORE] = (
            iv.transpose(1, 0, 2).reshape(TOK_PER_CORE, TOP_K)
            .astype(np.int32))
        sm = res["out_ssum"]                      # [16, 4*64]
        parts = np.empty((N_TILES, E), dtype=np.float32)
        for g in range(N_GROUPS):
            for m in range(TILES_PER_GROUP):
                parts[g * TILES_PER_GROUP + m] = (
                    sm[g * TILES_PER_GROUP + m, m * E:(m + 1) * E])
        ssum[c] = parts.sum(axis=0)

    scores_seq_mean = ssum.reshape(BSZ, 2, E).sum(axis=1) / np.float32(SEQ)
    idx_b = topk_idx.reshape(BSZ, SEQ * TOP_K)
    ce = np.stack(
        [np.bincount(idx_b[b], minlength=E) for b in range(BSZ)]
    ).astype(np.float32)
    ce = ce / np.float32(SEQ * TOP_K / E)
    aux_loss = np.float32((ce * scores_seq_mean).sum(axis=1).mean() * ALPHA)
    return topk_idx, topk_w, aux_loss


def kernel(hidden_states, weight):
    out = _run(hidden_states, weight)
    return _assemble(out.results)


# revision 14
# speedup vs baseline: 1.0292x; 1.0292x over previous
"""MoE gate (router) kernel for Trainium2, 8 NeuronCores.

Computes, for hidden_states [4, 4096, 2048] f32 and router weight [64, 2048] f32:
  logits = x @ w.T -> softmax over 64 experts -> top-2 (+ normalized weights)
  plus the seq_aux load-balancing loss.

Sharding: tokens (batch*seq = 16384) split contiguously across 8 cores
(2048 each); router weight replicated. Host pre-tiles each token shard into
the exact SBUF tile layout (group-major 512-token groups, 2-chunk DMA
pieces, bf16 hi/lo halves fused) so every DMA is a pure contiguous copy.
The bf16 hi/lo split gives the router GEMM bf16 PE rate at ~fp32 accuracy
(products are exact; only the fp32 psum accumulate rounds).

Kernel structure (per core): 512-token groups are pipelined - group g's
softmax/top-2 (DVE/ACT) overlaps group g+1's GEMM (PE). x loads alternate
between the two HWDGE rings (sync + scalar); outputs ride SWDGE (gpsimd).
A burst of dummy tiny matmuls at kernel start warms the PE clock (HAM)
during the first DMA wait.
"""

import os
import numpy as np
import ml_dtypes

import concourse.bass as bass
import concourse.mybir as mybir
import concourse.tile as tile
from concourse import bacc
from concourse.bass_utils import run_bass_kernel_spmd

F32 = mybir.dt.float32
BF16 = mybir.dt.bfloat16
F16 = mybir.dt.float16
F8E3 = mybir.dt.float8e3
I32 = mybir.dt.int32
U32 = mybir.dt.uint32
NP_BF16 = ml_dtypes.bfloat16

N_CORES = 8
BSZ = 4
SEQ = 4096
DIM = 2048
E = 64
TOP_K = 2
ALPHA = 0.01
N_TOKENS = BSZ * SEQ                 # 16384
TOK_PER_CORE = N_TOKENS // N_CORES   # 2048
P = 128
N_TILES = TOK_PER_CORE // P          # 16 token tiles per core
H_CHUNKS = DIM // P                  # 16 contraction chunks
N_GROUPS = 4                         # 512-token matmul groups
GTOK = TOK_PER_CORE // N_GROUPS      # 512
TILES_PER_GROUP = GTOK // P          # 4
KSPLIT = 4                           # h-chunks per x DMA piece
N_K = H_CHUNKS // KSPLIT             # 4 DMA pieces per group (per split)
LO_SCALE = 2048.0                    # fp8 lo-part scaling (f16x3 mode)

MODE = os.environ.get("BASS_MOE_MODE", "f16x3")
N_WARMUP = int(os.environ.get("BASS_MOE_WARMUP", "18"))


def _softmax_top2(nc, soft, pl, t, ex_all, rs_all, w_all, idx_all):
    """pl: PSUM [128, 64] logits for token tile t. Emits softmax+top2."""
    negmax = soft.tile([P, 1], F32, tag="negmax")
    nc.vector.tensor_reduce(
        negmax, pl, axis=mybir.AxisListType.X,
        op=mybir.AluOpType.max, negate=True,
    )
    ex_t = ex_all[:, t * E:(t + 1) * E]
    sumex = soft.tile([P, 1], F32, tag="sumex")
    nc.scalar.activation(
        ex_t, pl, mybir.ActivationFunctionType.Exp,
        bias=negmax, scale=1.0, accum_out=sumex,
    )
    nc.vector.reciprocal(rs_all[:, t:t + 1], sumex)

    m8 = soft.tile([P, 8], F32, tag="m8")
    nc.vector.max(out=m8, in_=ex_t)
    nc.vector.max_index(out=idx_all[:, 8 * t:8 * t + 8], in_max=m8,
                        in_values=ex_t)

    # m8[:,0] == 1.0 exactly (exp(max-max)); s12 = 1 + e2
    s12 = soft.tile([P, 1], F32, tag="s12")
    nc.vector.tensor_scalar_add(s12, m8[:, 1:2], 1.0)
    rs12 = soft.tile([P, 1], F32, tag="rs12")
    nc.vector.reciprocal(rs12, s12)
    nc.vector.tensor_scalar_mul(w_all[:, 2 * t:2 * t + 2], m8[:, 0:2], rs12)


def build_program(mode=MODE):
    nc = bacc.Bacc("TRN2", target_bir_lowering=False)

    if mode == "f32":
        x_specs = [("xt", F32)]
        w_specs = [("wt", F32)]
        terms = [(0, 0)]
    elif mode == "f16x3":
        # hi fp16, lo fp8(e3m4) scaled by LO_SCALE; w splits fp16 and the
        # lo-term stationary is wh/LO_SCALE so the scaling cancels exactly
        x_specs = [("xh", F16), ("xl", F8E3)]
        w_specs = [("wth", F16), ("wtl", F16), ("wths", F16)]
        terms = [(0, 0), (1, 0), (2, 1)]
    else:
        x_specs = [("xh", BF16), ("xl", BF16)]
        w_specs = [("wth", BF16), ("wtl", BF16)]
        terms = [(0, 0), (1, 0), (0, 1)]
        if mode == "bf16x4":
            terms.append((1, 1))

    # pre-tiled x: [group, kpiece, partition, chunk-in-piece, token]
    x_ins = [nc.dram_tensor(n, [N_GROUPS, N_K, P, KSPLIT, GTOK], dt,
                            kind="ExternalInput") for n, dt in x_specs]
    wt_ins = [nc.dram_tensor(n, [P, H_CHUNKS, E], dt, kind="ExternalInput")
              for n, dt in w_specs]
    ident_in = nc.dram_tensor("ident", [P, P], F32, kind="ExternalInput")

    out_w = nc.dram_tensor("out_w", [P, 2 * N_TILES], F32, kind="ExternalOutput")
    out_idx = nc.dram_tensor("out_idx", [P, 8 * N_TILES], U32,
                             kind="ExternalOutput")
    out_ssum = nc.dram_tensor("out_ssum", [N_TILES, TILES_PER_GROUP * E], F32,
                              kind="ExternalOutput")

    with tile.TileContext(nc) as tc:
        with (
            tc.tile_pool(name="consts", bufs=1) as consts,
            tc.tile_pool(name="xload", bufs=2) as xload,
            tc.tile_pool(name="lgsb", bufs=2) as lgsb,
            tc.tile_pool(name="soft", bufs=4) as soft,
            tc.tile_pool(name="acc", bufs=1) as acc,
            tc.tile_pool(name="pwu", bufs=1, space="PSUM") as pwu,
            tc.tile_pool(name="plg", bufs=2, space="PSUM") as plg,
            tc.tile_pool(name="plp", bufs=2, space="PSUM") as plp,
            tc.tile_pool(name="pst", bufs=2, space="PSUM") as pst,
        ):
            # ---- PE warmup: no-dep matmuls while first DMAs land ----
            # ~N_WARMUP x 213ns of PE busy; they sit in the PE's 64-deep
            # queue so they keep running even when the sequencer blocks on
            # the first data-dependent matmul.
            if N_WARMUP:
                wu_sb = consts.tile([P, 512], BF16)
                nc.vector.memset(wu_sb, 0.0)
                wu_ps = pwu.tile([8, 512], F32)
                for _ in range(N_WARMUP):
                    nc.tensor.matmul(wu_ps, lhsT=wu_sb[:, :8], rhs=wu_sb,
                                     start=True, stop=True,
                                     skip_group_check=True)

            # weights ride the scalar (ACT) HWDGE ring, host-pretiled so the
            # DMA is a contiguous copy; identity loads first on the sync ring
            ident_sb = consts.tile([P, P], F32)
            nc.sync.dma_start(ident_sb, ident_in[:, :])
            wt_sbs = []
            for wi, wt_in in enumerate(wt_ins):
                wsb = consts.tile([P, H_CHUNKS, E], w_specs[wi][1],
                                  tag=f"wt{wi}", name=f"wt{wi}")
                nc.scalar.dma_start(wsb, wt_in[:, :, :])
                wt_sbs.append(wsb)

            ex_all = acc.tile([P, N_TILES * E], F32)
            rs_all = acc.tile([P, N_TILES], F32)
            w_all = acc.tile([P, 2 * N_TILES], F32)
            idx_all = acc.tile([P, 8 * N_TILES], U32)

            lgT_live = {}

            def emit_gemm(g, dma_i):
                # this group's x pieces; rings balanced so each gets half of
                # hi and half of lo, consecutive pieces alternating
                xg = {}
                for xi, (xn, xdt) in enumerate(x_specs):
                    parts = []
                    for k in range(N_K):
                        xk = xload.tile([P, KSPLIT, GTOK], xdt,
                                        tag=f"x{xi}k{k}", name=f"x{xi}k{k}")
                        eng = nc.sync if ((k + xi + g) % 2 == 0) else nc.scalar
                        eng.dma_start(xk, x_ins[xi][g, k])
                        parts.append(xk)
                    xg[xi] = parts

                lgT = plg.tile([E, GTOK], F32, tag="lgT")
                for c in range(H_CHUNKS):
                    k, j = divmod(c, KSPLIT)
                    for ti, (wi, xi) in enumerate(terms):
                        nc.tensor.matmul(
                            lgT,
                            lhsT=wt_sbs[wi][:, c, :],
                            rhs=xg[xi][k][:, j, :],
                            start=(c == 0 and ti == 0),
                            stop=(c == H_CHUNKS - 1 and ti == len(terms) - 1),
                            skip_group_check=True,
                        )
                lgT_live[g] = lgT
                return dma_i

            def emit_softmax(g):
                # logits^T [64, 512] -> sbuf -> 4x transpose -> softmax
                lgT_sb = lgsb.tile([E, GTOK], F32, tag="lgT_sb")
                nc.scalar.copy(lgT_sb, lgT_live.pop(g))
                for j in range(TILES_PER_GROUP):
                    t = g * TILES_PER_GROUP + j
                    pl = plp.tile([P, E], F32, tag="logits")
                    nc.tensor.transpose(
                        pl, lgT_sb[:, j * P:(j + 1) * P], ident_sb[:E, :E],
                    )
                    _softmax_top2(nc, soft, pl, t, ex_all, rs_all, w_all,
                                  idx_all)

                # group aux-loss partials (diag-block trick):
                # out[m, :] = sum_tok rs[tok, g*4+m] * ex[tok, :]
                ps = pst.tile([TILES_PER_GROUP, TILES_PER_GROUP * E], F32,
                              tag="stats")
                nc.tensor.matmul(
                    ps,
                    lhsT=rs_all[:, g * TILES_PER_GROUP:(g + 1) * TILES_PER_GROUP],
                    rhs=ex_all[:, g * TILES_PER_GROUP * E:
                               (g + 1) * TILES_PER_GROUP * E],
                    start=True, stop=True,
                    skip_group_check=True,
                )
                ss = soft.tile([TILES_PER_GROUP, TILES_PER_GROUP * E], F32,
                               tag="ss")
                nc.scalar.copy(ss, ps)
                nc.sync.dma_start(
                    out_ssum[g * TILES_PER_GROUP:(g + 1) * TILES_PER_GROUP, :],
                    ss)

            # software pipeline: group g's softmax is emitted after group
            # g+1's GEMM so the PE stream never stalls on the lgT copy
            dma_i = 0
            for g in range(N_GROUPS):
                dma_i = emit_gemm(g, dma_i)
                if g >= 1:
                    emit_softmax(g - 1)
            emit_softmax(N_GROUPS - 1)

            nc.sync.dma_start(out_w[:, :], w_all[:, :])
            nc.sync.dma_start(out_idx[:, :], idx_all[:, :])

    if not nc.is_finalized():
        nc.finalize()
    return nc


_NC = {}


def _get_nc(mode=MODE):
    if mode not in _NC:
        _NC[mode] = build_program(mode)
    return _NC[mode]


def _pretile(xs):
    """[2048 tok, 2048 dim] -> [N_GROUPS, N_K, P, KSPLIT, GTOK] contiguous."""
    # (g, t, k, j, p) -> (g, k, p, j, t)
    r = xs.reshape(N_GROUPS, GTOK, N_K, KSPLIT, P)
    return np.ascontiguousarray(r.transpose(0, 2, 4, 3, 1))


def _prep_inputs(hidden_states, weight, mode=MODE):
    x = np.asarray(hidden_states, dtype=np.float32).reshape(N_TOKENS, DIM)
    w = np.asarray(weight, dtype=np.float32)
    ident = np.eye(P, dtype=np.float32)

    def wtile(arr):
        return np.ascontiguousarray(
            arr.T.reshape(H_CHUNKS, P, E).transpose(1, 0, 2))

    in_maps = []
    if mode == "f32":
        wt = wtile(w)
        for c in range(N_CORES):
            xs = x[c * TOK_PER_CORE:(c + 1) * TOK_PER_CORE, :]
            in_maps.append({"xt": _pretile(xs), "wt": wt, "ident": ident})
    elif mode == "f16x3":
        import ml_dtypes as mld
        wh = w.astype(np.float16)
        wl = (w - wh.astype(np.float32)).astype(np.float16)
        whs = (wh.astype(np.float32) / LO_SCALE).astype(np.float16)
        wth, wtl, wths = wtile(wh), wtile(wl), wtile(whs)
        for c in range(N_CORES):
            xs = x[c * TOK_PER_CORE:(c + 1) * TOK_PER_CORE, :]
            xh = xs.astype(np.float16)
            xls = ((xs - xh.astype(np.float32)) * LO_SCALE).astype(
                mld.float8_e3m4)
            in_maps.append({
                "xh": _pretile(xh), "xl": _pretile(xls),
                "wth": wth, "wtl": wtl, "wths": wths, "ident": ident,
            })
    else:
        wh = w.astype(NP_BF16)
        wl = (w - wh.astype(np.float32)).astype(NP_BF16)
        wth, wtl = wtile(wh), wtile(wl)
        for c in range(N_CORES):
            xs = x[c * TOK_PER_CORE:(c + 1) * TOK_PER_CORE, :]
            xh = xs.astype(NP_BF16)
            xl = (xs - xh.astype(np.float32)).astype(NP_BF16)
            in_maps.append({
                "xh": _pretile(xh), "xl": _pretile(xl),
                "wth": wth, "wtl": wtl, "ident": ident,
            })
    return in_maps


def _run(hidden_states, weight, trace=False, mode=MODE, tmpdir=None):
    in_maps = _prep_inputs(hidden_states, weight, mode)
    nc = _get_nc(mode)
    kw = {}
    if trace:
        kw["trace"] = True
        if tmpdir:
            kw["tmpdir"] = tmpdir
    return run_bass_kernel_spmd(nc, in_maps, list(range(N_CORES)), **kw)


def _assemble(results):
    topk_w = np.empty((N_TOKENS, TOP_K), dtype=np.float32)
    topk_idx = np.empty((N_TOKENS, TOP_K), dtype=np.int32)
    ssum = np.empty((N_CORES, E), dtype=np.float32)
    for c, res in enumerate(results):
        wv = res["out_w"].reshape(P, N_TILES, TOP_K)
        iv = res["out_idx"].reshape(P, N_TILES, 8)[:, :, :TOP_K]
        base = c * TOK_PER_CORE
        topk_w[base:base + TOK_PER_CORE] = (
            wv.transpose(1, 0, 2).reshape(TOK_PER_CORE, TOP_K))
        topk_idx[base:base + TOK_PER_CORE] = (
            iv.transpose(1, 0, 2).reshape(TOK_PER_CORE, TOP_K)
            .astype(np.int32))
        sm = res["out_ssum"]                      # [16, 4*64]
        parts = np.empty((N_TILES, E), dtype=np.float32)
        for g in range(N_GROUPS):
            for m in range(TILES_PER_GROUP):
                parts[g * TILES_PER_GROUP + m] = (
                    sm[g * TILES_PER_GROUP + m, m * E:(m + 1) * E])
        ssum[c] = parts.sum(axis=0)

    scores_seq_mean = ssum.reshape(BSZ, 2, E).sum(axis=1) / np.float32(SEQ)
    idx_b = topk_idx.reshape(BSZ, SEQ * TOP_K)
    ce = np.stack(
        [np.bincount(idx_b[b], minlength=E) for b in range(BSZ)]
    ).astype(np.float32)
    ce = ce / np.float32(SEQ * TOP_K / E)
    aux_loss = np.float32((ce * scores_seq_mean).sum(axis=1).mean() * ALPHA)
    return topk_idx, topk_w, aux_loss


def kernel(hidden_states, weight):
    out = _run(hidden_states, weight)
    return _assemble(out.results)
